# revision 1
# baseline (speedup 1.0000x reference)
"""Trainium2 Bass kernel for nn_CLIP_3v3d_brats (dense_cnn head + gated 1x1 conv).

Sharding: 8 cores = batch(2) x 4 D-slabs of `pred`. The dominant einsum
logits[b,k,:] = sum_c effw[b,k,c]*pred[b,c,:] runs as a block-diagonal
float32r matmul (4 position groups -> K=128, M=12, ~1 cycle/row).

GAP head: conv+global-mean collapse to x_feat = W2d @ S where S are 27
strided window sums of relu(groupnorm(x_e)). Head work is sharded by
CHANNELS (16 ch/core = exactly 2 GroupNorm groups, so stats stay
core-local); window sums use a host-gathered (channel, offset-group)
partition packing with ReLU+normalize+window-sum fused into single ACT
instructions via accum_out. ONE AllReduce total (x_feat partials); a
dep-free warmup collective absorbs part of the mesh cold-start. The
collective fabric has a fixed ~60-100us cold-start from kernel launch
(cross-core skew), so the stream prefetches ~13.5MB of pred into SBUF
during that window; the remaining stream runs DMA-saturated.
"""
import sys
import types

sys.path.insert(0, "/opt/trn_rl_repo")

import numpy as np

# Register the NTFF profile hook the agent image's antenv lacks (only
# needed when TRACE is enabled; harmless otherwise).
try:
    import antenv.axon_hooks  # noqa: F401
except ImportError:
    try:
        import trn_agent_boot.trn_boot as _tb

        _hooks = types.ModuleType("antenv.axon_hooks")
        _the_hook = _tb._ntff_profile_via_ctypes("/opt/axon/libaxon_pjrt.so")
        _hooks.get_axon_ntff_profile_hook = lambda: _the_hook
        _hooks.set_axon_ntff_profile_hook = lambda h: None
        sys.modules["antenv.axon_hooks"] = _hooks
    except Exception:
        pass

from concourse import bacc, tile, mybir
from concourse.bass_utils import run_bass_kernel_spmd

f32 = mybir.dt.float32
f32r = mybir.dt.float32r
AF = mybir.ActivationFunctionType
ALU = mybir.AluOpType

N_CORES = 8
B = 2
K = 3
EPS = 1e-5
G = 4                      # position groups interleaved on partitions
NPOS = 221184              # positions per core slab: 24*96*96
NG = NPOS // G             # 55296
COLS = 4608                # stream iteration columns (9 matmuls of 512)
NITER = NG // COLS         # 12
NMM = COLS // 512          # 9
NSLOT = 4                  # window-sum slots per core
NWIN = 1331                # 11^3 window positions per offset
NSLAB = 1728               # 3*24*24 stats-slab positions per batch
NGRP_ELEMS = 8 * 13824     # elements per (batch, group) for GN stats

TRACE = False
LAST_EXEC_NS = None
_CACHE = {}


def _build_program():
    nc = bacc.Bacc("TRN2", target_bir_lowering=False, debug=False,
                   num_devices=N_CORES)

    def din(name, shape, dt=f32):
        return nc.dram_tensor(name, shape, dt, kind="ExternalInput").ap()

    pred_s = din("pred_s", [NITER, 128, COLS], f32r)
    xe_slab_d = din("xe_slab", [128, B * NSLAB])
    xg_d = din("xg", [128, B * NSLOT * NWIN])
    w2dt_d = din("w2dt", [128, NSLOT * 256])
    gstat_d = din("gstat", [128, 2])
    gexp_d = din("gexp", [2, 128])
    gnw_d = din("gnw", [128, 2])
    w_cfT_d = din("w_cfT", [128, 2 * 512])
    bcf6_d = din("bcf6", [6, 4 * 128])
    id6_d = din("id6", [6, 6])
    w_cT_d = din("w_cT", [128, 4 * 256])
    bcT_d = din("bcT", [128, 2])
    w_a1T_d = din("w_a1T", [128, 2 * 16])
    ba1_d = din("ba1", [16, 1])
    w_a2T_d = din("w_a2T", [16, 32])
    ba2_d = din("ba2", [32, 1])
    wseg6_d = din("wseg6", [32, 6])
    msel_d = din("msel", [32, 2])
    gapbT_d = din("gapbT", [128, 4])
    ones6_d = din("ones6", [128, 6])
    bseg12_d = din("bseg12", [12, 1])

    out_d = nc.dram_tensor("out_s", [12, NG], f32,
                           kind="ExternalOutput").ap()

    with tile.TileContext(nc) as tc:
        with tc.tile_pool(name="small", bufs=1) as sp, \
             tc.tile_pool(name="pred", bufs=6) as pp, \
             tc.tile_pool(name="outp", bufs=2) as op, \
             tc.tile_pool(name="hps", bufs=3, space="PSUM") as hps, \
             tc.tile_pool(name="sps", bufs=4, space="PSUM") as sps, \
             tc.tile_pool(name="dram", bufs=1, space="DRAM") as dram:
          with tc.tile_pool(name="headbig", bufs=1) as hb, \
               tc.tile_pool(name="scratch", bufs=2) as scp:

            # ---- head constant loads (emitted first: priority on DMA) ----
            xe_slab = hb.tile([128, B * NSLAB], f32)
            nc.sync.dma_start(xe_slab[:], xe_slab_d[:])
            gstat = sp.tile([128, 2], f32)
            nc.sync.dma_start(gstat[:], gstat_d[:])
            gexp = sp.tile([2, 128], f32)
            nc.sync.dma_start(gexp[:], gexp_d[:])
            gnw = sp.tile([128, 2], f32)
            nc.sync.dma_start(gnw[:], gnw_d[:])
            xgs = []
            for col in range(B * NSLOT):
                xgt = scp.tile([128, NWIN], f32, tag="xgs", bufs=2)
                nc.sync.dma_start(
                    xgt[:], xg_d[:, col * NWIN:(col + 1) * NWIN])
                xgs.append(xgt)
            w2dt = hb.tile([128, NSLOT * 256], f32)
            nc.sync.dma_start(w2dt[:], w2dt_d[:])
            w_cfT = hb.tile([128, 2 * 512], f32)
            nc.sync.dma_start(w_cfT[:], w_cfT_d[:])
            bcf6 = sp.tile([6, 4 * 128], f32)
            nc.sync.dma_start(bcf6[:], bcf6_d[:])
            id6 = sp.tile([6, 6], f32)
            nc.sync.dma_start(id6[:], id6_d[:])
            w_cT = hb.tile([128, 4 * 256], f32)
            nc.sync.dma_start(w_cT[:], w_cT_d[:])
            bcT = sp.tile([128, 2], f32)
            nc.sync.dma_start(bcT[:], bcT_d[:])
            w_a1T = sp.tile([128, 2 * 16], f32)
            nc.sync.dma_start(w_a1T[:], w_a1T_d[:])
            ba1 = sp.tile([16, 1], f32)
            nc.sync.dma_start(ba1[:], ba1_d[:])
            w_a2T = sp.tile([16, 32], f32)
            nc.sync.dma_start(w_a2T[:], w_a2T_d[:])
            ba2 = sp.tile([32, 1], f32)
            nc.sync.dma_start(ba2[:], ba2_d[:])
            wseg6 = sp.tile([32, 6], f32)
            nc.sync.dma_start(wseg6[:], wseg6_d[:])
            msel = sp.tile([32, 2], f32)
            nc.sync.dma_start(msel[:], msel_d[:])
            gapbT = sp.tile([128, 4], f32)
            nc.sync.dma_start(gapbT[:], gapbT_d[:])
            ones6 = sp.tile([128, 6], f32)
            nc.sync.dma_start(ones6[:], ones6_d[:])
            bseg12 = sp.tile([12, 1], f32)
            nc.sync.dma_start(bseg12[:], bseg12_d[:])

            # ---- collective-fabric warmup (no data deps) ----
            warm_in = dram.tile([2, 2], f32)
            warm_out = dram.tile([2, 2], f32)
            nc.gpsimd.collective_compute(
                "AllReduce", ALU.add,
                replica_groups=[list(range(N_CORES))],
                ins=[warm_in.opt()], outs=[warm_out.opt()])

            # ---- GN stats (core-local: 16 channels = 2 full groups) ----
            stat4 = sp.tile([128, 4], f32)  # cols: 2*b + (0=sum, 1=sumsq)
            for b in range(B):
                sl = xe_slab[:, b * NSLAB:(b + 1) * NSLAB]
                st_sc = scp.tile([128, NSLAB], f32, tag="sc", bufs=1)
                nc.scalar.activation(st_sc[:], sl, AF.Copy,
                                     accum_out=stat4[:, 2 * b:2 * b + 1])
                st_sc2 = scp.tile([128, NSLAB], f32, tag="sc", bufs=1)
                nc.scalar.activation(st_sc2[:], sl, AF.Square,
                                     accum_out=stat4[:, 2 * b + 1:2 * b + 2])

            # group-sum via mask matmul: [2, 4] (both groups are core-local)
            g4 = hps.tile([2, 4], f32, tag="hps")
            nc.tensor.matmul(g4[:], gstat[:], stat4[:], start=True, stop=True)
            gsum = sp.tile([2, 4], f32)
            nc.vector.tensor_copy(gsum[:], g4[:])

            # mu(neg), rsqrt(var+eps) per (group, b) -> mr4 [2,4]
            mr4 = sp.tile([2, 4], f32)  # cols: -mu0, -mu1, rs0, rs1
            nc.scalar.mul(mr4[:, 0:2], gsum[:, 0:4:2], -1.0 / NGRP_ELEMS)
            ex2 = sp.tile([2, 2], f32)
            nc.scalar.mul(ex2[:], gsum[:, 1:4:2], 1.0 / NGRP_ELEMS)
            musq = sp.tile([2, 2], f32)
            nc.vector.tensor_mul(musq[:], mr4[:, 0:2], mr4[:, 0:2])
            var = sp.tile([2, 2], f32)
            nc.vector.tensor_sub(var[:], ex2[:], musq[:])
            vare = sp.tile([2, 2], f32)
            nc.vector.tensor_scalar_add(vare[:], var[:], float(EPS))
            sd = sp.tile([2, 2], f32)
            nc.scalar.activation(sd[:], vare[:], AF.Sqrt)
            nc.vector.reciprocal(mr4[:, 2:4], sd[:])

            # expand groups -> (c,og) partitions: chmr [128,4]
            ch4 = hps.tile([128, 4], f32, tag="hps")
            nc.tensor.matmul(ch4[:], gexp[:], mr4[:], start=True, stop=True)
            chmr = sp.tile([128, 4], f32)
            nc.vector.tensor_copy(chmr[:], ch4[:])
            # scale_c = rs*gamma ; bias_c = beta + (-mu)*scale
            scale = sp.tile([128, 2], f32)
            nc.vector.tensor_scalar_mul(scale[:], chmr[:, 2:4], gnw[:, 0:1])
            nmus = sp.tile([128, 2], f32)
            nc.vector.tensor_mul(nmus[:], chmr[:, 0:2], scale[:])
            bias = sp.tile([128, 2], f32)
            nc.vector.tensor_scalar_add(bias[:], nmus[:], gnw[:, 1:2])

            # ---- fused relu-normalize + window-sum into S4 [128, 8] ----
            S4 = sp.tile([128, B * NSLOT], f32)
            for b in range(B):
                for s in range(NSLOT):
                    col = b * NSLOT + s
                    rl_sc = scp.tile([128, NWIN], f32, tag="sc", bufs=1)
                    nc.scalar.activation(
                        rl_sc[:], xgs[col][:], AF.Relu,
                        bias=bias[:, b:b + 1], scale=scale[:, b:b + 1],
                        accum_out=S4[:, col:col + 1])

            # ---- x_feat partials: xfT chunks [128, 2] via W2dT matmuls ----
            xfs = sp.tile([128, 4], f32)  # cols: oc*2 + b
            for oc in range(2):
                xfp = hps.tile([128, 2], f32, tag="hps")
                for s in range(NSLOT):
                    nc.tensor.matmul(
                        xfp[:],
                        w2dt[:, s * 256 + oc * 128: s * 256 + oc * 128 + 128],
                        S4[:, s:s + NSLOT + 1:NSLOT],
                        start=(s == 0), stop=(s == NSLOT - 1))
                nc.vector.tensor_copy(xfs[:, oc * 2:oc * 2 + 2], xfp[:])

            ar2_in = dram.tile([128, 4], f32)
            ar2_out = dram.tile([128, 4], f32)
            nc.gpsimd.dma_start(ar2_in[:], xfs[:])
            nc.gpsimd.collective_compute(
                "AllReduce", ALU.add,
                replica_groups=[list(range(N_CORES))],
                ins=[ar2_in.opt()], outs=[ar2_out.opt()])
            xfr = sp.tile([128, 4], f32)
            nc.gpsimd.dma_start(xfr[:], ar2_out[:])
            xfb = sp.tile([128, 4], f32)
            nc.vector.tensor_add(xfb[:], xfr[:], gapbT[:])

            # ---- xcT for feature half: [128, 12] cols pc*6 + (3b+k) ----
            xcT = sp.tile([128, 12], f32)
            for pc in range(2):
                for b in range(B):
                    nc.vector.tensor_scalar_mul(
                        xcT[:, pc * 6 + 3 * b: pc * 6 + 3 * b + 3],
                        ones6[:, 0:3],
                        xfb[:, pc * 2 + b: pc * 2 + b + 1])

            # ---- MLP1: p6T = relu(Wx @ x_feat + (We@emb + b_cf)).T ----
            p6T = sp.tile([128, 4 * 6], f32)
            for oc in range(4):
                p1 = hps.tile([128, 6], f32, tag="hps")
                for pc in range(2):
                    nc.tensor.matmul(
                        p1[:],
                        w_cfT[:, pc * 512 + oc * 128: pc * 512 + oc * 128 + 128],
                        xcT[:, pc * 6:pc * 6 + 6],
                        start=(pc == 0), stop=False)
                nc.tensor.matmul(p1[:], bcf6[:, oc * 128:(oc + 1) * 128],
                                 id6[:], start=False, stop=True)
                nc.scalar.activation(p6T[:, oc * 6:oc * 6 + 6], p1[:], AF.Relu)

            # ---- MLP2: c6T [128, 2*6] ----
            c6T = sp.tile([128, 2 * 6], f32)
            for oc in range(2):
                c1 = hps.tile([128, 6], f32, tag="hps")
                for pc in range(4):
                    nc.tensor.matmul(
                        c1[:],
                        w_cT[:, pc * 256 + oc * 128: pc * 256 + oc * 128 + 128],
                        p6T[:, pc * 6:pc * 6 + 6],
                        start=(pc == 0), stop=(pc == 3))
                nc.scalar.activation(c6T[:, oc * 6:oc * 6 + 6], c1[:],
                                     AF.Identity, bias=bcT[:, oc:oc + 1])

            # ---- MLP3: hT [16, 6] ----
            h1 = hps.tile([16, 6], f32, tag="hps")
            for pc in range(2):
                nc.tensor.matmul(h1[:], w_a1T[:, pc * 16:pc * 16 + 16],
                                 c6T[:, pc * 6:pc * 6 + 6],
                                 start=(pc == 0), stop=(pc == 1))
            hT = sp.tile([16, 6], f32)
            nc.scalar.activation(hT[:], h1[:], AF.Relu, bias=ba1[:, 0:1])

            # ---- MLP4: gT [32, 6] = sigmoid(...) ----
            g1 = hps.tile([32, 6], f32, tag="hps")
            nc.tensor.matmul(g1[:], w_a2T[:], hT[:], start=True, stop=True)
            gT = sp.tile([32, 6], f32)
            nc.scalar.activation(gT[:], g1[:], AF.Sigmoid, bias=ba2[:, 0:1])

            # ---- effw + batch select + block-diagonal lhsT [128, 12] ----
            effT = sp.tile([32, 6], f32)
            nc.vector.tensor_mul(effT[:], gT[:], wseg6[:])
            selL = sp.tile([32, 3], f32)
            nc.vector.tensor_scalar_mul(selL[:], effT[:, 0:3], msel[:, 0:1])
            selR = sp.tile([32, 3], f32)
            nc.vector.tensor_scalar_mul(selR[:], effT[:, 3:6], msel[:, 1:2])
            effB = sp.tile([32, 3], f32)
            nc.vector.tensor_add(effB[:], selL[:], selR[:])

            bd = sp.tile([128, 12], f32r)
            nc.vector.memset(bd[:].bitcast(mybir.dt.uint32), 0)
            for g in range(G):
                nc.sync.dma_start(bd[32 * g:32 * g + 32, 3 * g:3 * g + 3],
                                  effB[:].bitcast(f32r))

          # ---- main stream: 12 x (one 2.25MB DMA -> 9 matmuls -> out) ----
          for t in range(NITER):
            pt = pp.tile([128, COLS], f32r, tag="pt")
            for g in range(G):
                nc.sync.dma_start(pt[32 * g:32 * g + 32, :],
                                  pred_s[t, 32 * g:32 * g + 32, :])
            so = op.tile([12, COLS], f32, tag="so")
            for m in range(NMM):
                po = sps.tile([12, 512], f32, tag="po")
                nc.tensor.matmul(po[:], bd[:], pt[:, m * 512:(m + 1) * 512],
                                 start=True, stop=True)
                if m % 2 == 0:
                    nc.scalar.activation(so[:, m * 512:(m + 1) * 512], po[:],
                                         AF.Identity, bias=bseg12[:, 0:1])
                else:
                    nc.vector.tensor_scalar_add(
                        so[:, m * 512:(m + 1) * 512], po[:], bseg12[:, 0:1])
            nc.gpsimd.dma_start(out_d[:, t * COLS:(t + 1) * COLS], so[:])

    nc.compile()
    return nc


def _prep_shared(inp):
    """Host-side weight transposes shared by all cores."""
    gn_g = np.asarray(inp["gn_g"], np.float32)
    gn_b = np.asarray(inp["gn_b"], np.float32)
    gap_b = np.asarray(inp["gap_b"], np.float32)
    w_cf = np.asarray(inp["w_cf"], np.float32)
    b_cf = np.asarray(inp["b_cf"], np.float32)
    w_c = np.asarray(inp["w_c"], np.float32)
    b_c = np.asarray(inp["b_c"], np.float32)
    w_a1 = np.asarray(inp["w_a1"], np.float32)
    b_a1 = np.asarray(inp["b_a1"], np.float32)
    w_a2 = np.asarray(inp["w_a2"], np.float32)
    b_a2 = np.asarray(inp["b_a2"], np.float32)
    emb = np.asarray(inp["emb"], np.float32)
    w_seg = np.asarray(inp["w_seg"], np.float32)
    b_seg = np.asarray(inp["b_seg"], np.float32)

    p = np.arange(128)
    gstat = (p[:, None] // 64 == np.arange(2)[None, :]).astype(np.float32)
    gexp = np.ascontiguousarray(gstat.T)

    # x-half of w_cf, transposed: [128, 2*512]
    wx = w_cf[:, 0:256].T                            # [256, 512]
    w_cfT = np.concatenate(
        [wx[128 * pc:128 * (pc + 1), :] for pc in range(2)], axis=1)
    # constant-folded emb-half + bias: bcf6[r, o] = b_cf[o] + We @ emb
    j = np.arange(6)
    bcf6 = np.ascontiguousarray(
        b_cf[None, :] + emb[j % 3] @ w_cf[:, 256:512].T)  # [6, 512]
    id6 = np.eye(6, dtype=np.float32)
    w_cT = np.concatenate(
        [w_c.T[128 * pc:128 * (pc + 1), :] for pc in range(4)], axis=1)
    bcT = np.ascontiguousarray(b_c.reshape(2, 128).T)
    w_a1T = np.concatenate(
        [w_a1.T[128 * pc:128 * (pc + 1), :] for pc in range(2)], axis=1)
    ba1 = b_a1.reshape(16, 1)
    w_a2T = np.ascontiguousarray(w_a2.T)
    ba2 = b_a2.reshape(32, 1)

    wseg6 = np.ascontiguousarray(w_seg[j % 3, :].T)
    gapbT = np.ascontiguousarray(
        np.repeat(gap_b.reshape(2, 128).T, 2, axis=1))  # cols oc*2+b
    ones6 = np.ones((128, 6), np.float32)
    bseg12 = np.tile(b_seg, 4).reshape(12, 1)

    return dict(gstat=gstat, gexp=gexp, w_cfT=w_cfT, bcf6=bcf6, id6=id6,
                w_cT=w_cT, bcT=bcT, w_a1T=w_a1T, ba1=ba1, w_a2T=w_a2T,
                ba2=ba2, wseg6=wseg6, gapbT=gapbT, ones6=ones6,
                bseg12=bseg12)


def kernel(**inputs):
    global LAST_EXEC_NS
    x_e = np.asarray(inputs["x_e"], np.float32)
    pred = np.asarray(inputs["pred"], np.float32)
    gap_w = np.asarray(inputs["gap_w"], np.float32)
    gn_g = np.asarray(inputs["gn_g"], np.float32)
    gn_b = np.asarray(inputs["gn_b"], np.float32)

    shared = _prep_shared(inputs)
    shared = {k: np.ascontiguousarray(v, dtype=np.float32)
              for k, v in shared.items()}

    # (og, s) -> conv offset table, identical on every core
    offs = [(4 * og + s) % 27 for og in range(8) for s in range(NSLOT)]
    cnt = np.bincount(np.array(offs), minlength=27).astype(np.float32)
    w2 = gap_w.reshape(256, 128, 27)

    # all 27 strided windows of x_e, gathered once: [27, B, 128, NWIN]
    wins = np.empty((27, B, 128, NWIN), np.float32)
    for o in range(27):
        kd, kw, kh = o // 9, (o // 3) % 3, o % 3
        win = x_e[:, :, kd:kd + 21:2, kw:kw + 21:2, kh:kh + 21:2]
        wins[o] = win.reshape(B, 128, NWIN)

    in_maps = []
    for r in range(N_CORES):
        b, dq = divmod(r, 4)
        m = dict(shared)
        ch = slice(16 * r, 16 * r + 16)

        ps = pred[b, :, dq * 24:(dq + 1) * 24]          # [32,24,96,96]
        ps = ps.reshape(32, G, NITER, COLS).transpose(2, 1, 0, 3)
        m["pred_s"] = np.ascontiguousarray(ps.reshape(NITER, 128, COLS))

        # stats slab: partitions (c:16, dchunk:8), cols b*1728 + pos
        sl = x_e[:, ch].reshape(B, 16, 8, NSLAB)
        m["xe_slab"] = np.ascontiguousarray(
            sl.transpose(1, 2, 0, 3).reshape(128, -1))

        # window gather: partitions (c:16, og:8), cols (b, s, pos)
        xgr = np.empty((16, 8, B, NSLOT, NWIN), np.float32)
        w2dt = np.empty((16, 8, NSLOT, 256), np.float32)
        for og in range(8):
            for sidx in range(NSLOT):
                o = offs[og * NSLOT + sidx]
                xgr[:, og, :, sidx, :] = wins[o][:, ch].transpose(1, 0, 2)
                w2dt[:, og, sidx, :] = (
                    w2[:, ch, o].T / np.float32(1331.0 * cnt[o]))
        m["xg"] = np.ascontiguousarray(xgr.reshape(128, -1))
        m["w2dt"] = np.ascontiguousarray(w2dt.reshape(128, -1))

        # per-(c,og) gamma/beta
        m["gnw"] = np.ascontiguousarray(
            np.stack([np.repeat(gn_g[ch], 8), np.repeat(gn_b[ch], 8)],
                     axis=1))

        msel = np.zeros((32, 2), np.float32)
        msel[:, b] = 1.0
        m["msel"] = msel
        in_maps.append(m)

    if "nc" not in _CACHE:
        _CACHE["nc"] = _build_program()
    nc = _CACHE["nc"]

    res = run_bass_kernel_spmd(nc, in_maps, list(range(N_CORES)),
                               trace=TRACE)
    LAST_EXEC_NS = res.exec_time_ns

    out = np.empty((B, K, 96, 96, 96), np.float32)
    for r in range(N_CORES):
        b, dq = divmod(r, 4)
        o = res.results[r]["out_s"]                      # [12, NG]
        o = o.reshape(G, K, NG).transpose(1, 0, 2).reshape(K, NPOS)
        out[b, :, dq * 24:(dq + 1) * 24] = o.reshape(K, 24, 96, 96)
    return out

